# revision 22
# baseline (speedup 1.0000x reference)
"""Trainium2 Bass kernel for nn_DarcyResidual (P=256, B=128, 8 NeuronCores).

Math (reference):
    a = (x0 + 1.5) / 0.2,  p = (x1 + 0.9) / 115
    residual = -a*(p_d00 + p_d11) - a_d0*p_d0 - a_d1*p_d1 - 1
2nd-order central differences inside, 2nd-order one-sided at borders,
h = 1/256 on both axes.

Folded all-bf16 form computed here (G = 65536/92).  The host pre-scales
channel 0 to A' = -G*(X0 + 1.5) so that, with raw (unscaled) integer
stencils,
    residual = A'*U4 + S1*R1 + C1a*C1p - 1
      U4  = 4*(rowD2raw(X1) + colD2raw(X1))
      R1  = rowD1raw(X1),  S1 = rowD1raw(A')     (= -G*rowD1raw(X0))
      C1p = colD1raw(X1),  C1a = colD1raw(A')
and no trailing affine is needed.

Layout per core (16 images): SBUF [partition = row-within-128-block,
free = (row-block k:2, image b:2, col j:256)], 8 chunks of 2 images.
Row (d0) derivatives are TensorE matmuls with BLOCK-DIAGONAL banded
stencil matrices only (the off-diagonal blocks are dropped); the two
image rows this corrupts (127, 128) are recomputed exactly by a small
"fixup" pipeline in a (ch, row, image)-partition layout fed by a tiny
host-gathered tensor with ghost columns that make the one-sided column
stencils central.  U4 accumulates in PSUM: W_R2 = 4*(D2 - 2I) diagonal
blocks plus two 4I matmuls on +-1-column-shifted rhs views.  ScalarE
evacuates U4 to bf16; DVE does the bf16 2x-mode products; GPSIMD takes
the gradient-term combine.  Border columns j=0,255 come from a dense
edge pipeline (full-matrix matmuls over a host-pregathered 8-column
tensor) patched into the result tiles before output DMA.  Output rows
127/128 ride a separate tiny output tensor merged on the host.
"""

import numpy as np

P = 256
B = 128
NCORES = 8
BPC = B // NCORES          # images per core = 16
CHUNKS = 8
BCH = BPC // CHUNKS        # images per chunk = 2
FCH = 2 * BCH * P          # chunk free size = 1024
GAMMA = 65536.0 / 92.0

_cache = {}


def _build_mats():
    D1 = np.zeros((P, P), dtype=np.float64)
    D2 = np.zeros((P, P), dtype=np.float64)
    for i in range(1, P - 1):
        D1[i, i - 1] = -1.0
        D1[i, i + 1] = 1.0
        D2[i, i - 1] = 1.0
        D2[i, i] = -2.0
        D2[i, i + 1] = 1.0
    D1[0, 0:3] = [-3.0, 4.0, -1.0]
    D1[P - 1, P - 3:P] = [1.0, -4.0, 3.0]
    D2[0, 0:4] = [2.0, -5.0, 4.0, -1.0]
    D2[P - 1, P - 4:P] = [-1.0, 4.0, -5.0, 2.0]
    return D1, D2


def _weights():
    """[128, 13, 128] stacked lhsT blocks (bf16).
    0,1: D1 diag blocks; 2,3: 4*(D2-2I) diag blocks; 4: 4I;
    5-8: D1 full blocks (edge); 9-12: 4*D2 full blocks (edge)."""
    import ml_dtypes
    D1, D2 = _build_mats()
    WR2 = 4.0 * (D2 - 2.0 * np.eye(P))
    WR2E = 4.0 * D2
    wts = np.zeros((128, 13, 128), dtype=np.float32)

    def blk(Wm, m, kb):
        return Wm[m * 128:(m + 1) * 128, kb * 128:(kb + 1) * 128].T

    wts[:, 0, :] = blk(D1, 0, 0)
    wts[:, 1, :] = blk(D1, 1, 1)
    wts[:, 2, :] = blk(WR2, 0, 0)
    wts[:, 3, :] = blk(WR2, 1, 1)
    wts[:, 4, :] = 4.0 * np.eye(128)
    for m in range(2):
        for kb in range(2):
            wts[:, 5 + m * 2 + kb, :] = blk(D1, m, kb)
            wts[:, 9 + m * 2 + kb, :] = blk(WR2E, m, kb)
    return wts.astype(ml_dtypes.bfloat16)


def _build_program():
    from concourse import bacc
    import concourse.mybir as mybir
    from concourse.tile import TileContext

    f32 = mybir.dt.float32
    bf16 = mybir.dt.bfloat16
    ADD = mybir.AluOpType.add
    SUB = mybir.AluOpType.subtract
    MUL = mybir.AluOpType.mult
    COPY = mybir.ActivationFunctionType.Copy

    nc = bacc.Bacc("TRN2", target_bir_lowering=False, debug=False,
                   num_devices=NCORES)
    xin = nc.dram_tensor("xin", [128, 2, 2, BPC, P], bf16, kind="ExternalInput")
    xe = nc.dram_tensor("xe", [128, 2, 2, BPC, 8], bf16, kind="ExternalInput")
    xfix = nc.dram_tensor("xfix", [32, 7, P + 2], bf16, kind="ExternalInput")
    wts = nc.dram_tensor("wts", [128, 13, 128], bf16, kind="ExternalInput")
    yout = nc.dram_tensor("yout", [128, 2, BPC, P], bf16, kind="ExternalOutput")
    yfix = nc.dram_tensor("yfix", [32, P], bf16, kind="ExternalOutput")

    with TileContext(nc) as tc:
        with (
            tc.tile_pool(name="const", bufs=1) as cpool,
            tc.tile_pool(name="edge", bufs=1) as epool,
            tc.tile_pool(name="work", bufs=2) as pool,
            tc.tile_pool(name="psum", bufs=2, space="PSUM") as pp,
        ):
            wt = cpool.tile([128, 13, 128], bf16)
            nc.sync.dma_start(out=wt[:], in_=wts[:])
            Xfx = epool.tile([32, 7, P + 2], bf16)
            nc.sync.dma_start(out=Xfx[:], in_=xfix[:])
            # chunk-0 main input: flat [A' | X1] with 2-elem pads at the
            # outer ends only (A' at [2:FCH+2], X1 at [FCH+2:2*FCH+2])
            XA0 = pool.tile([128, 2 * FCH + 4], bf16, tag="xa", bufs=3)
            nc.sync.dma_start(
                out=XA0[:, 2:2 * FCH + 2].rearrange(
                    "p (c k b j) -> p c k b j", c=2, k=2, b=BCH),
                in_=xin[:, :, :, 0:BCH, :])
            Xe = epool.tile([128, 2, 2, BPC, 8], bf16)
            nc.sync.dma_start(out=Xe[:], in_=xe[:])

            def W(i):
                return wt[:, i, :]

            stt = nc.vector.scalar_tensor_tensor
            A0 = 2          # A' channel base offset in XA
            X1 = FCH + 2    # X1 channel base offset in XA

            def chunk_mm(c, XA):
                """Matmul + PSUM-evacuation phase of one chunk."""
                u16 = pool.tile([128, FCH], bf16, name=f"u16_{c}",
                                tag="u16", bufs=3)
                s16 = pool.tile([128, FCH], bf16, name=f"s16_{c}",
                                tag="s16", bufs=3)
                r16 = pool.tile([128, FCH], bf16, name=f"r16_{c}",
                                tag="r16", bufs=3)
                R1b = pp.tile([128, 1024], f32, name=f"r1_{c}", tag="r1")
                for m in range(2):
                    lo = m * (BCH * P)
                    hi = lo + BCH * P
                    S1m = pp.tile([128, 512], f32, name=f"s1_{c}_{m}",
                                  tag="s1")
                    U4m = pp.tile([128, 512], f32, name=f"u4_{c}_{m}",
                                  tag="u4")
                    nc.tensor.matmul(R1b[:, lo:hi], W(m),
                                     XA[:, X1 + lo:X1 + hi],
                                     start=True, stop=True)
                    nc.tensor.matmul(S1m[:], W(m), XA[:, A0 + lo:A0 + hi],
                                     start=True, stop=True)
                    nc.tensor.matmul(U4m[:], W(2 + m),
                                     XA[:, X1 + lo:X1 + hi],
                                     start=True, stop=False)
                    nc.tensor.matmul(U4m[:], W(4),
                                     XA[:, X1 + lo - 1:X1 + hi - 1],
                                     start=False, stop=False)
                    nc.tensor.matmul(U4m[:], W(4),
                                     XA[:, X1 + lo + 1:X1 + hi + 1],
                                     start=False, stop=True)
                    osl = slice(m * (BCH * P), (m + 1) * (BCH * P))
                    nc.scalar.copy(out=u16[:, osl], in_=U4m[:])
                    nc.scalar.copy(out=s16[:, osl], in_=S1m[:])
                nc.scalar.copy(out=r16[:], in_=R1b[:])
                return u16, s16, r16

            def chunk_ew_a(c, XA):
                """Early DVE phase: column stencils (input-only deps)."""
                c1b = pool.tile([128, 2, FCH], bf16, name=f"c1b_{c}",
                                tag="c1b", bufs=3)
                t3 = pool.tile([128, FCH], bf16, name=f"t3_{c}",
                               tag="t3", bufs=3)
                # shifted col-d1 stencils: slot t holds value for col t+1
                nc.vector.tensor_sub(
                    c1b[:],
                    XA[:, 4:2 * FCH + 4].rearrange("p (c f) -> p c f", c=2),
                    XA[:, 2:2 * FCH + 2].rearrange("p (c f) -> p c f", c=2))
                nc.vector.tensor_mul(t3[:], c1b[:, 0, :], c1b[:, 1, :])
                return t3

            def chunk_ew_b(c, XA, t3, u16, s16, r16, rese):
                """Late DVE phase + border patch + output DMA."""
                b0c = c * BCH
                t2 = pool.tile([128, FCH], bf16, name=f"t2_{c}",
                               tag="t2", bufs=3)
                g = pool.tile([128, FCH], bf16, name=f"g_{c}",
                              tag="g", bufs=3)
                tm = pool.tile([128, FCH], bf16, name=f"tm_{c}",
                               tag="tm", bufs=3)
                res = pool.tile([128, 2, BCH, P], bf16, name=f"res_{c}",
                                tag="res", bufs=3)
                resfl = res.rearrange("p k b j -> p (k b j)")
                nc.vector.tensor_mul(t2[:], s16[:], r16[:])
                nc.vector.tensor_mul(tm[:], XA[:, A0:A0 + FCH], u16[:])
                nc.vector.tensor_add(g[:, 1:FCH], t2[:, 1:FCH],
                                     t3[:, 0:FCH - 1])
                nc.vector.tensor_add(resfl[:, 1:FCH], tm[:, 1:FCH],
                                     g[:, 1:FCH])

                # patch border cols from the edge pipeline, then ship out
                esrc = (rese.rearrange("p (k b) e -> p k b e", k=2)
                        [:, :, b0c:b0c + BCH, :])
                nc.gpsimd.tensor_copy(out=res[:, :, :, 0:P:P - 1], in_=esrc)
                nc.gpsimd.dma_start(out=yout[:, :, b0c:b0c + BCH, :],
                                    in_=res[:])

            # ---------- fixup pipeline (rows 127,128; all cols) -------------
            # Runs first: its tiny input lands early and its ops fill the
            # DVE startup bubble.  Xfx holds, per output-row slot
            # p = (r-127)*16 + b, seven pre-aligned operand planes:
            # 0=X1 center (col-d1 ghosts), 1=X1 center (col-d2 ghosts),
            # 2=A' center (col-d1 ghosts), 3=X1 row r-1, 4=X1 row r+1,
            # 5=A' row r-1, 6=A' row r+1.  Ghost cols at 0/257 turn the
            # one-sided column stencils at j=0/255 into central ones.
            def ft(name):
                return epool.tile([32, P], bf16, name=name)

            tX1 = Xfx[:, 0, :]
            tX2 = Xfx[:, 1, :]
            tA1 = Xfx[:, 2, :]
            R1f, S1f, shr, shc = ft("R1f"), ft("S1f"), ft("shr"), ft("shc")
            sb, u4q, tmf = ft("sb"), ft("u4q"), ft("tmf")
            c1pf, c1af, t3f, t2f = ft("c1pf"), ft("c1af"), ft("t3f"), ft("t2f")
            gf, resf = ft("gf"), epool.tile([32, P], bf16, name="resf")
            nc.gpsimd.tensor_sub(R1f[:], Xfx[:, 4, 1:P + 1],
                                 Xfx[:, 3, 1:P + 1])
            nc.gpsimd.tensor_sub(S1f[:], Xfx[:, 6, 1:P + 1],
                                 Xfx[:, 5, 1:P + 1])
            nc.gpsimd.tensor_mul(t2f[:], R1f[:], S1f[:])
            nc.vector.tensor_add(shr[:], Xfx[:, 4, 1:P + 1],
                                 Xfx[:, 3, 1:P + 1])
            nc.vector.tensor_add(shc[:], tX2[:, 0:P], tX2[:, 2:P + 2])
            nc.vector.tensor_add(sb[:], shr[:], shc[:])
            stt(u4q[:], tX1[:, 1:P + 1], -4.0, sb[:], MUL, ADD)
            stt(tmf[:], u4q[:], 4.0, tA1[:, 1:P + 1], MUL, MUL)
            nc.vector.tensor_sub(c1pf[:], tX1[:, 2:P + 2], tX1[:, 0:P])
            nc.vector.tensor_sub(c1af[:], tA1[:, 2:P + 2], tA1[:, 0:P])
            nc.vector.tensor_mul(t3f[:], c1pf[:], c1af[:])
            nc.vector.tensor_add(gf[:], t2f[:], t3f[:])
            nc.vector.tensor_add(resf[:], tmf[:], gf[:])
            nc.gpsimd.dma_start(out=yfix[:], in_=resf[:])

            # ---------- chunk-0 matmul + early-DVE (ahead of the edge) -----
            mm0 = chunk_mm(0, XA0)
            t3_0 = chunk_ew_a(0, XA0)

            # ---------- edge pipeline (cols j=0,255; all 256 rows) ----------
            R2e = pp.tile([128, 512], f32, tag="u4")
            R1e = pp.tile([128, 512], f32, tag="r1")
            S1e = pp.tile([128, 512], f32, tag="s1")
            X1ef = Xe[:, 1].rearrange("p k b c -> p (k b c)")
            X0ef = Xe[:, 0].rearrange("p k b c -> p (k b c)")
            for m in range(2):
                osl = slice(m * 128, (m + 1) * 128)
                for kb in range(2):
                    isl = slice(kb * 128, (kb + 1) * 128)
                    st, sp = kb == 0, kb == 1
                    nc.tensor.matmul(R1e[:, osl], W(5 + m * 2 + kb),
                                     X1ef[:, isl], start=st, stop=sp)
                    nc.tensor.matmul(S1e[:, osl], W(5 + m * 2 + kb),
                                     X0ef[:, isl], start=st, stop=sp)
                    nc.tensor.matmul(R2e[:, osl], W(9 + m * 2 + kb),
                                     X1ef[:, isl], start=st, stop=sp)

            E1 = Xe[:, 1].rearrange("p k b c -> p (k b) c")   # [128, 32, 8]
            E0 = Xe[:, 0].rearrange("p k b c -> p (k b) c")

            def et(name):
                return epool.tile([128, 2 * BPC, 2], f32, name=name)

            # paired forward diffs: half 0 = j=0 side, half 1 = j=255 side
            a1, b1, c1 = et("a1"), et("b1"), et("c1")
            a0, b0 = et("a0"), et("b0")
            nc.vector.tensor_sub(a1[:], E1[:, :, 1:8:6], E1[:, :, 0:7:6])
            nc.vector.tensor_sub(b1[:], E1[:, :, 2:7:4], E1[:, :, 1:6:4])
            nc.vector.tensor_sub(c1[:], E1[:, :, 3:6:2], E1[:, :, 2:5:2])
            nc.vector.tensor_sub(a0[:], E0[:, :, 1:8:6], E0[:, :, 0:7:6])
            nc.vector.tensor_sub(b0[:], E0[:, :, 2:7:4], E0[:, :, 1:6:4])

            q, Z = et("q"), et("Z")
            C1pe, C1ae = et("C1pe"), et("C1ae")
            stt(q[:], b1[:], 3.0, c1[:], MUL, SUB)      # 3b - c
            stt(Z[:], a1[:], -2.0, q[:], MUL, ADD)      # -2a + 3b - c
            stt(C1pe[:], a1[:], 3.0, b1[:], MUL, SUB)   # 3a - b
            stt(C1ae[:], a0[:], 3.0, b0[:], MUL, SUB)

            RP2 = R2e[:, 0:256].rearrange("p (g c) -> p g c", c=8)
            RP1 = R1e[:, 0:256].rearrange("p (g c) -> p g c", c=8)
            U4e, tme, t2e = et("U4e"), et("tme"), et("t2e")
            stt(U4e[:, :, 0:1], Z[:, :, 0:1], 4.0, RP2[:, :, 0:1], MUL, ADD)
            stt(U4e[:, :, 1:2], Z[:, :, 1:2], -4.0, RP2[:, :, 7:8], MUL, ADD)

            Scpe = epool.tile([128, 2 * BPC, 8], f32)
            nc.scalar.copy(out=Scpe.rearrange("p g c -> p (g c)"),
                           in_=S1e[:, 0:256])
            nc.vector.tensor_mul(tme[:], E0[:, :, 0:8:7], U4e[:])
            nc.vector.tensor_mul(t2e[:], Scpe[:, :, 0:8:7], RP1[:, :, 0:8:7])
            nc.vector.tensor_mul(C1ae[:], C1ae[:], C1pe[:])   # t3e in-place
            nc.vector.tensor_add(tme[:], tme[:], t2e[:])
            rese = epool.tile([128, 2 * BPC, 2], bf16)
            nc.vector.tensor_add(rese[:], tme[:], C1ae[:])

            # ---------- main pipeline, 8 chunks of 2 images -----------------
            for c in range(CHUNKS):
                if c == 0:
                    XA = XA0
                    u16, s16, r16 = mm0
                    t3 = t3_0
                else:
                    XA = pool.tile([128, 2 * FCH + 4], bf16,
                                   name=f"xa_{c}", tag="xa", bufs=3)
                    nc.sync.dma_start(
                        out=XA[:, 2:2 * FCH + 2].rearrange(
                            "p (c k b j) -> p c k b j", c=2, k=2, b=BCH),
                        in_=xin[:, :, :, c * BCH:(c + 1) * BCH, :])
                    u16, s16, r16 = chunk_mm(c, XA)
                    t3 = chunk_ew_a(c, XA)
                chunk_ew_b(c, XA, t3, u16, s16, r16, rese)

    nc.compile()
    return nc


def _get_program():
    if "nc" not in _cache:
        _cache["nc"] = _build_program()
        _cache["wts"] = _weights()
    return _cache["nc"], _cache["wts"]


def _shard_inputs(x0_pred):
    import ml_dtypes
    x = np.ascontiguousarray(np.asarray(x0_pred, dtype=np.float32))
    _, wts = _get_program()
    in_maps = []
    cols = [0, 1, 2, 3, P - 4, P - 3, P - 2, P - 1]
    for i in range(NCORES):
        shard = x[i * BPC:(i + 1) * BPC]                      # [16,2,256,256]
        ap = np.empty_like(shard)
        ap[:, 0] = -GAMMA * (shard[:, 0] + 1.5)               # A'
        ap[:, 1] = shard[:, 1]                                # X1
        arr = ap.reshape(BPC, 2, 2, 128, P).transpose(3, 1, 2, 0, 4)
        arr = np.ascontiguousarray(arr).astype(ml_dtypes.bfloat16)
        xe = np.ascontiguousarray(arr[:, :, :, :, cols])
        # fixup tensor: 7 operand planes for output rows r in {127,128},
        # partition slot p = (r-127)*16 + b
        def rows(ch, r0, r1):
            return ap[:, ch, (r0, r1), :].transpose(1, 0, 2).reshape(32, P)

        xc = rows(1, 127, 128)      # X1 center
        ac = rows(0, 127, 128)      # A' center
        xfixv = np.zeros((32, 7, P + 2), dtype=np.float32)
        for plane, fr in ((0, xc), (1, xc), (2, ac)):
            xfixv[:, plane, 1:P + 1] = fr
        for plane, fr in ((0, xc), (2, ac)):                  # col-d1 ghosts
            xfixv[:, plane, 0] = 3 * fr[:, 0] - 3 * fr[:, 1] + fr[:, 2]
            xfixv[:, plane, P + 1] = (3 * fr[:, P - 1] - 3 * fr[:, P - 2]
                                      + fr[:, P - 3])
        xfixv[:, 1, 0] = 4 * xc[:, 0] - 6 * xc[:, 1] + 4 * xc[:, 2] - xc[:, 3]
        xfixv[:, 1, P + 1] = (4 * xc[:, P - 1] - 6 * xc[:, P - 2]
                              + 4 * xc[:, P - 3] - xc[:, P - 4])
        xfixv[:, 3, 1:P + 1] = rows(1, 126, 127)              # X1 r-1
        xfixv[:, 4, 1:P + 1] = rows(1, 128, 129)              # X1 r+1
        xfixv[:, 5, 1:P + 1] = rows(0, 126, 127)              # A' r-1
        xfixv[:, 6, 1:P + 1] = rows(0, 128, 129)              # A' r+1
        in_maps.append({"xin": arr, "xe": xe, "wts": wts,
                        "xfix": xfixv.astype(ml_dtypes.bfloat16)})
    return in_maps


def _unshard(results):
    outs = []
    for i in range(NCORES):
        y = np.asarray(results[i]["yout"], dtype=np.float32)  # [128,2,16,256]
        img = y.transpose(2, 1, 0, 3).reshape(BPC, 1, P, P)
        yf = np.asarray(results[i]["yfix"], dtype=np.float32)  # [32, 256]
        yf = yf.reshape(2, BPC, P)
        img[:, 0, 127, :] = yf[0]
        img[:, 0, 128, :] = yf[1]
        outs.append(img)
    out = np.concatenate(outs, axis=0)
    out -= 1.0                    # source term f_s == 1, folded here
    return np.ascontiguousarray(out)


def _run(x0_pred, trace=False, tmpdir=None):
    import time
    from concourse.bass_utils import run_bass_kernel_spmd
    nc = _get_program()[0]
    in_maps = _shard_inputs(x0_pred)
    try:
        res = run_bass_kernel_spmd(nc, in_maps, list(range(NCORES)),
                                   trace=trace, tmpdir=tmpdir)
    except Exception:
        # transient NRT execution failures have been observed; one retry
        time.sleep(2.0)
        res = run_bass_kernel_spmd(nc, in_maps, list(range(NCORES)),
                                   trace=trace, tmpdir=tmpdir)
    return _unshard(res.results), res


def kernel(x0_pred):
    out, _ = _run(x0_pred, trace=False)
    return out


# revision 24
# speedup vs baseline: 1.0858x; 1.0858x over previous
"""Trainium2 Bass kernel for nn_DarcyResidual (P=256, B=128, 8 NeuronCores).

Math (reference):
    a = (x0 + 1.5) / 0.2,  p = (x1 + 0.9) / 115
    residual = -a*(p_d00 + p_d11) - a_d0*p_d0 - a_d1*p_d1 - 1
2nd-order central differences inside, 2nd-order one-sided at borders,
h = 1/256 on both axes.

Folded all-bf16 form computed here (G = 65536/92).  The host pre-scales
channel 0 to A' = -G*(X0 + 1.5) so that, with raw (unscaled) integer
stencils,
    residual = A'*U4 + S1*R1 + C1a*C1p - 1
      U4  = 4*(rowD2raw(X1) + colD2raw(X1))
      R1  = rowD1raw(X1),  S1 = rowD1raw(A')     (= -G*rowD1raw(X0))
      C1p = colD1raw(X1),  C1a = colD1raw(A')
and no trailing affine is needed.

Layout per core (16 images): SBUF [partition = row-within-128-block,
free = (row-block k:2, image b:2, col j:256)], 8 chunks of 2 images.
Row (d0) derivatives are TensorE matmuls with BLOCK-DIAGONAL banded
stencil matrices only (the off-diagonal blocks are dropped); the two
image rows this corrupts (127, 128) are recomputed exactly by a small
"fixup" pipeline in a (ch, row, image)-partition layout fed by a tiny
host-gathered tensor with ghost columns that make the one-sided column
stencils central.  U4 accumulates in PSUM: W_R2 = 4*(D2 - 2I) diagonal
blocks plus two 4I matmuls on +-1-column-shifted rhs views.  ScalarE
evacuates U4 to bf16; DVE does the bf16 2x-mode products; GPSIMD takes
the gradient-term combine.  Border columns j=0,255 come from a dense
edge pipeline (full-matrix matmuls over a host-pregathered 8-column
tensor) patched into the result tiles before output DMA.  Output rows
127/128 ride a separate tiny output tensor merged on the host.
"""

import numpy as np

P = 256
B = 128
NCORES = 8
BPC = B // NCORES          # images per core = 16
CHUNKS = 8
BCH = BPC // CHUNKS        # images per chunk = 2
FCH = 2 * BCH * P          # chunk free size = 1024
GAMMA = 65536.0 / 92.0

_cache = {}


def _build_mats():
    D1 = np.zeros((P, P), dtype=np.float64)
    D2 = np.zeros((P, P), dtype=np.float64)
    for i in range(1, P - 1):
        D1[i, i - 1] = -1.0
        D1[i, i + 1] = 1.0
        D2[i, i - 1] = 1.0
        D2[i, i] = -2.0
        D2[i, i + 1] = 1.0
    D1[0, 0:3] = [-3.0, 4.0, -1.0]
    D1[P - 1, P - 3:P] = [1.0, -4.0, 3.0]
    D2[0, 0:4] = [2.0, -5.0, 4.0, -1.0]
    D2[P - 1, P - 4:P] = [-1.0, 4.0, -5.0, 2.0]
    return D1, D2


def _weights():
    """[128, 13, 128] stacked lhsT blocks (bf16).
    0,1: D1 diag blocks; 2,3: 4*(D2-2I) diag blocks; 4: 4I;
    5-8: D1 full blocks (edge); 9-12: 4*D2 full blocks (edge)."""
    import ml_dtypes
    D1, D2 = _build_mats()
    WR2 = 4.0 * (D2 - 2.0 * np.eye(P))
    WR2E = 4.0 * D2
    wts = np.zeros((128, 13, 128), dtype=np.float32)

    def blk(Wm, m, kb):
        return Wm[m * 128:(m + 1) * 128, kb * 128:(kb + 1) * 128].T

    wts[:, 0, :] = blk(D1, 0, 0)
    wts[:, 1, :] = blk(D1, 1, 1)
    wts[:, 2, :] = blk(WR2, 0, 0)
    wts[:, 3, :] = blk(WR2, 1, 1)
    wts[:, 4, :] = 4.0 * np.eye(128)
    for m in range(2):
        for kb in range(2):
            wts[:, 5 + m * 2 + kb, :] = blk(D1, m, kb)
            wts[:, 9 + m * 2 + kb, :] = blk(WR2E, m, kb)
    return wts.astype(ml_dtypes.bfloat16)


def _build_program():
    from concourse import bacc
    import concourse.mybir as mybir
    from concourse.tile import TileContext

    f32 = mybir.dt.float32
    bf16 = mybir.dt.bfloat16
    ADD = mybir.AluOpType.add
    SUB = mybir.AluOpType.subtract
    MUL = mybir.AluOpType.mult
    COPY = mybir.ActivationFunctionType.Copy

    nc = bacc.Bacc("TRN2", target_bir_lowering=False, debug=False,
                   num_devices=NCORES)
    xin = nc.dram_tensor("xin", [128, 2, 2, BPC, P], bf16, kind="ExternalInput")
    xe = nc.dram_tensor("xe", [128, 2, 2, BPC, 8], bf16, kind="ExternalInput")
    xfix = nc.dram_tensor("xfix", [32, 7, P + 2], bf16, kind="ExternalInput")
    wts = nc.dram_tensor("wts", [128, 13, 128], bf16, kind="ExternalInput")
    yout = nc.dram_tensor("yout", [128, 2, BPC, P], bf16, kind="ExternalOutput")
    yfix = nc.dram_tensor("yfix", [32, P], bf16, kind="ExternalOutput")
    yedge = nc.dram_tensor("yedge", [128, 2 * BPC, 2], bf16,
                           kind="ExternalOutput")

    with TileContext(nc) as tc:
        with (
            tc.tile_pool(name="const", bufs=1) as cpool,
            tc.tile_pool(name="edge", bufs=1) as epool,
            tc.tile_pool(name="work", bufs=2) as pool,
            tc.tile_pool(name="psum", bufs=2, space="PSUM") as pp,
        ):
            wt = cpool.tile([128, 13, 128], bf16)
            nc.sync.dma_start(out=wt[:], in_=wts[:])
            Xfx = epool.tile([32, 7, P + 2], bf16)
            nc.sync.dma_start(out=Xfx[:], in_=xfix[:])
            # chunk-0 main input: flat [A' | X1] with 2-elem pads at the
            # outer ends only (A' at [2:FCH+2], X1 at [FCH+2:2*FCH+2])
            XA0 = pool.tile([128, 2 * FCH + 4], bf16, tag="xa", bufs=3)
            nc.sync.dma_start(
                out=XA0[:, 2:2 * FCH + 2].rearrange(
                    "p (c k b j) -> p c k b j", c=2, k=2, b=BCH),
                in_=xin[:, :, :, 0:BCH, :])
            Xe = epool.tile([128, 2, 2, BPC, 8], bf16)
            nc.sync.dma_start(out=Xe[:], in_=xe[:])

            def W(i):
                return wt[:, i, :]

            stt = nc.vector.scalar_tensor_tensor
            A0 = 2          # A' channel base offset in XA
            X1 = FCH + 2    # X1 channel base offset in XA

            def chunk_mm(c, XA):
                """Matmul + PSUM-evacuation phase of one chunk."""
                u16 = pool.tile([128, FCH], bf16, name=f"u16_{c}",
                                tag="u16", bufs=3)
                s16 = pool.tile([128, FCH], bf16, name=f"s16_{c}",
                                tag="s16", bufs=3)
                r16 = pool.tile([128, FCH], bf16, name=f"r16_{c}",
                                tag="r16", bufs=3)
                R1b = pp.tile([128, 1024], f32, name=f"r1_{c}", tag="r1")
                for m in range(2):
                    lo = m * (BCH * P)
                    hi = lo + BCH * P
                    S1m = pp.tile([128, 512], f32, name=f"s1_{c}_{m}",
                                  tag="s1")
                    U4m = pp.tile([128, 512], f32, name=f"u4_{c}_{m}",
                                  tag="u4")
                    nc.tensor.matmul(R1b[:, lo:hi], W(m),
                                     XA[:, X1 + lo:X1 + hi],
                                     start=True, stop=True)
                    nc.tensor.matmul(S1m[:], W(m), XA[:, A0 + lo:A0 + hi],
                                     start=True, stop=True)
                    nc.tensor.matmul(U4m[:], W(2 + m),
                                     XA[:, X1 + lo:X1 + hi],
                                     start=True, stop=False)
                    nc.tensor.matmul(U4m[:], W(4),
                                     XA[:, X1 + lo - 1:X1 + hi - 1],
                                     start=False, stop=False)
                    nc.tensor.matmul(U4m[:], W(4),
                                     XA[:, X1 + lo + 1:X1 + hi + 1],
                                     start=False, stop=True)
                    osl = slice(m * (BCH * P), (m + 1) * (BCH * P))
                    nc.scalar.copy(out=u16[:, osl], in_=U4m[:])
                    nc.scalar.copy(out=s16[:, osl], in_=S1m[:])
                nc.scalar.copy(out=r16[:], in_=R1b[:])
                return u16, s16, r16

            def chunk_ew_a(c, XA):
                """Early DVE phase: column stencils (input-only deps)."""
                c1b = pool.tile([128, 2, FCH], bf16, name=f"c1b_{c}",
                                tag="c1b", bufs=3)
                t3 = pool.tile([128, FCH], bf16, name=f"t3_{c}",
                               tag="t3", bufs=3)
                # shifted col-d1 stencils: slot t holds value for col t+1
                nc.vector.tensor_sub(
                    c1b[:],
                    XA[:, 4:2 * FCH + 4].rearrange("p (c f) -> p c f", c=2),
                    XA[:, 2:2 * FCH + 2].rearrange("p (c f) -> p c f", c=2))
                nc.vector.tensor_mul(t3[:], c1b[:, 0, :], c1b[:, 1, :])
                return t3

            def chunk_ew_b(c, XA, t3, u16, s16, r16):
                """Late DVE phase + output DMA (border cols left garbage;
                host overwrites them from yedge)."""
                b0c = c * BCH
                t2 = pool.tile([128, FCH], bf16, name=f"t2_{c}",
                               tag="t2", bufs=3)
                g = pool.tile([128, FCH], bf16, name=f"g_{c}",
                              tag="g", bufs=3)
                tm = pool.tile([128, FCH], bf16, name=f"tm_{c}",
                               tag="tm", bufs=3)
                res = pool.tile([128, 2, BCH, P], bf16, name=f"res_{c}",
                                tag="res", bufs=3)
                resfl = res.rearrange("p k b j -> p (k b j)")
                nc.vector.tensor_mul(t2[:], s16[:], r16[:])
                nc.vector.tensor_mul(tm[:], XA[:, A0:A0 + FCH], u16[:])
                nc.vector.tensor_add(g[:, 1:FCH], t2[:, 1:FCH],
                                     t3[:, 0:FCH - 1])
                nc.vector.tensor_add(resfl[:, 1:FCH], tm[:, 1:FCH],
                                     g[:, 1:FCH])
                nc.gpsimd.dma_start(out=yout[:, :, b0c:b0c + BCH, :],
                                    in_=res[:])

            # ---------- fixup pipeline (rows 127,128; all cols) -------------
            # Runs first: its tiny input lands early and its ops fill the
            # DVE startup bubble.  Xfx holds, per output-row slot
            # p = (r-127)*16 + b, seven pre-aligned operand planes:
            # 0=X1 center (col-d1 ghosts), 1=X1 center (col-d2 ghosts),
            # 2=A' center (col-d1 ghosts), 3=X1 row r-1, 4=X1 row r+1,
            # 5=A' row r-1, 6=A' row r+1.  Ghost cols at 0/257 turn the
            # one-sided column stencils at j=0/255 into central ones.
            def ft(name):
                return epool.tile([32, P], bf16, name=name)

            tX1 = Xfx[:, 0, :]
            tX2 = Xfx[:, 1, :]
            tA1 = Xfx[:, 2, :]
            R1f, S1f, shr, shc = ft("R1f"), ft("S1f"), ft("shr"), ft("shc")
            sb, u4q, tmf = ft("sb"), ft("u4q"), ft("tmf")
            c1pf, c1af, t3f, t2f = ft("c1pf"), ft("c1af"), ft("t3f"), ft("t2f")
            gf, resf = ft("gf"), epool.tile([32, P], bf16, name="resf")
            nc.gpsimd.tensor_sub(R1f[:], Xfx[:, 4, 1:P + 1],
                                 Xfx[:, 3, 1:P + 1])
            nc.gpsimd.tensor_sub(S1f[:], Xfx[:, 6, 1:P + 1],
                                 Xfx[:, 5, 1:P + 1])
            nc.gpsimd.tensor_mul(t2f[:], R1f[:], S1f[:])
            nc.vector.tensor_add(shr[:], Xfx[:, 4, 1:P + 1],
                                 Xfx[:, 3, 1:P + 1])
            nc.vector.tensor_add(shc[:], tX2[:, 0:P], tX2[:, 2:P + 2])
            nc.vector.tensor_add(sb[:], shr[:], shc[:])
            stt(u4q[:], tX1[:, 1:P + 1], -4.0, sb[:], MUL, ADD)
            stt(tmf[:], u4q[:], 4.0, tA1[:, 1:P + 1], MUL, MUL)
            nc.vector.tensor_sub(c1pf[:], tX1[:, 2:P + 2], tX1[:, 0:P])
            nc.vector.tensor_sub(c1af[:], tA1[:, 2:P + 2], tA1[:, 0:P])
            nc.vector.tensor_mul(t3f[:], c1pf[:], c1af[:])
            nc.vector.tensor_add(gf[:], t2f[:], t3f[:])
            nc.vector.tensor_add(resf[:], tmf[:], gf[:])
            nc.gpsimd.dma_start(out=yfix[:], in_=resf[:])

            # ---------- edge pipeline part A: input-only DVE ops ------------
            E1 = Xe[:, 1].rearrange("p k b c -> p (k b) c")   # [128, 32, 8]
            E0 = Xe[:, 0].rearrange("p k b c -> p (k b) c")

            def et(name):
                return epool.tile([128, 2 * BPC, 2], f32, name=name)

            # paired forward diffs: half 0 = j=0 side, half 1 = j=255 side
            a1, b1, c1 = et("a1"), et("b1"), et("c1")
            a0, b0 = et("a0"), et("b0")
            nc.vector.tensor_sub(a1[:], E1[:, :, 1:8:6], E1[:, :, 0:7:6])
            nc.vector.tensor_sub(b1[:], E1[:, :, 2:7:4], E1[:, :, 1:6:4])
            nc.vector.tensor_sub(c1[:], E1[:, :, 3:6:2], E1[:, :, 2:5:2])
            nc.vector.tensor_sub(a0[:], E0[:, :, 1:8:6], E0[:, :, 0:7:6])
            nc.vector.tensor_sub(b0[:], E0[:, :, 2:7:4], E0[:, :, 1:6:4])
            q, Z = et("q"), et("Z")
            C1pe, C1ae = et("C1pe"), et("C1ae")
            stt(q[:], b1[:], 3.0, c1[:], MUL, SUB)      # 3b - c
            stt(Z[:], a1[:], -2.0, q[:], MUL, ADD)      # -2a + 3b - c
            stt(C1pe[:], a1[:], 3.0, b1[:], MUL, SUB)   # 3a - b
            stt(C1ae[:], a0[:], 3.0, b0[:], MUL, SUB)
            nc.vector.tensor_mul(C1ae[:], C1ae[:], C1pe[:])   # t3e in-place

            # ---------- chunk-0 matmul + early-DVE ---------------------------
            mm0 = chunk_mm(0, XA0)
            t3_0 = chunk_ew_a(0, XA0)

            # ---------- edge matmuls (after chunk-0's) ----------------------
            R2e = pp.tile([128, 512], f32, tag="u4")
            R1e = pp.tile([128, 512], f32, tag="r1")
            S1e = pp.tile([128, 512], f32, tag="s1")
            X1ef = Xe[:, 1].rearrange("p k b c -> p (k b c)")
            X0ef = Xe[:, 0].rearrange("p k b c -> p (k b c)")
            for m in range(2):
                osl = slice(m * 128, (m + 1) * 128)
                for kb in range(2):
                    isl = slice(kb * 128, (kb + 1) * 128)
                    st, sp = kb == 0, kb == 1
                    nc.tensor.matmul(R1e[:, osl], W(5 + m * 2 + kb),
                                     X1ef[:, isl], start=st, stop=sp)
                    nc.tensor.matmul(S1e[:, osl], W(5 + m * 2 + kb),
                                     X0ef[:, isl], start=st, stop=sp)
                    nc.tensor.matmul(R2e[:, osl], W(9 + m * 2 + kb),
                                     X1ef[:, isl], start=st, stop=sp)
            Scpe = epool.tile([128, 2 * BPC, 8], f32)
            nc.scalar.copy(out=Scpe.rearrange("p g c -> p (g c)"),
                           in_=S1e[:, 0:256])

            # ---------- chunk-0 late DVE ------------------------------------
            chunk_ew_b(0, XA0, t3_0, *mm0)

            # ---------- edge pipeline part B: PSUM combine -> yedge ---------
            RP2 = R2e[:, 0:256].rearrange("p (g c) -> p g c", c=8)
            RP1 = R1e[:, 0:256].rearrange("p (g c) -> p g c", c=8)
            U4e, tme, t2e = et("U4e"), et("tme"), et("t2e")
            stt(U4e[:, :, 0:1], Z[:, :, 0:1], 4.0, RP2[:, :, 0:1], MUL, ADD)
            stt(U4e[:, :, 1:2], Z[:, :, 1:2], -4.0, RP2[:, :, 7:8], MUL, ADD)
            nc.vector.tensor_mul(tme[:], E0[:, :, 0:8:7], U4e[:])
            nc.vector.tensor_mul(t2e[:], Scpe[:, :, 0:8:7], RP1[:, :, 0:8:7])
            nc.vector.tensor_add(tme[:], tme[:], t2e[:])
            rese = epool.tile([128, 2 * BPC, 2], bf16)
            nc.vector.tensor_add(rese[:], tme[:], C1ae[:])
            nc.gpsimd.dma_start(out=yedge[:], in_=rese[:])

            # ---------- main pipeline, 8 chunks of 2 images -----------------
            for c in range(1, CHUNKS):
                XA = pool.tile([128, 2 * FCH + 4], bf16,
                               name=f"xa_{c}", tag="xa", bufs=3)
                nc.sync.dma_start(
                    out=XA[:, 2:2 * FCH + 2].rearrange(
                        "p (c k b j) -> p c k b j", c=2, k=2, b=BCH),
                    in_=xin[:, :, :, c * BCH:(c + 1) * BCH, :])
                u16, s16, r16 = chunk_mm(c, XA)
                t3 = chunk_ew_a(c, XA)
                chunk_ew_b(c, XA, t3, u16, s16, r16)

    nc.compile()
    return nc


def _get_program():
    if "nc" not in _cache:
        _cache["nc"] = _build_program()
        _cache["wts"] = _weights()
    return _cache["nc"], _cache["wts"]


def _shard_inputs(x0_pred):
    import ml_dtypes
    x = np.ascontiguousarray(np.asarray(x0_pred, dtype=np.float32))
    _, wts = _get_program()
    in_maps = []
    cols = [0, 1, 2, 3, P - 4, P - 3, P - 2, P - 1]
    for i in range(NCORES):
        shard = x[i * BPC:(i + 1) * BPC]                      # [16,2,256,256]
        ap = np.empty_like(shard)
        ap[:, 0] = -GAMMA * (shard[:, 0] + 1.5)               # A'
        ap[:, 1] = shard[:, 1]                                # X1
        arr = ap.reshape(BPC, 2, 2, 128, P).transpose(3, 1, 2, 0, 4)
        arr = np.ascontiguousarray(arr).astype(ml_dtypes.bfloat16)
        xe = np.ascontiguousarray(arr[:, :, :, :, cols])
        # fixup tensor: 7 operand planes for output rows r in {127,128},
        # partition slot p = (r-127)*16 + b
        def rows(ch, r0, r1):
            return ap[:, ch, (r0, r1), :].transpose(1, 0, 2).reshape(32, P)

        xc = rows(1, 127, 128)      # X1 center
        ac = rows(0, 127, 128)      # A' center
        xfixv = np.zeros((32, 7, P + 2), dtype=np.float32)
        for plane, fr in ((0, xc), (1, xc), (2, ac)):
            xfixv[:, plane, 1:P + 1] = fr
        for plane, fr in ((0, xc), (2, ac)):                  # col-d1 ghosts
            xfixv[:, plane, 0] = 3 * fr[:, 0] - 3 * fr[:, 1] + fr[:, 2]
            xfixv[:, plane, P + 1] = (3 * fr[:, P - 1] - 3 * fr[:, P - 2]
                                      + fr[:, P - 3])
        xfixv[:, 1, 0] = 4 * xc[:, 0] - 6 * xc[:, 1] + 4 * xc[:, 2] - xc[:, 3]
        xfixv[:, 1, P + 1] = (4 * xc[:, P - 1] - 6 * xc[:, P - 2]
                              + 4 * xc[:, P - 3] - xc[:, P - 4])
        xfixv[:, 3, 1:P + 1] = rows(1, 126, 127)              # X1 r-1
        xfixv[:, 4, 1:P + 1] = rows(1, 128, 129)              # X1 r+1
        xfixv[:, 5, 1:P + 1] = rows(0, 126, 127)              # A' r-1
        xfixv[:, 6, 1:P + 1] = rows(0, 128, 129)              # A' r+1
        in_maps.append({"xin": arr, "xe": xe, "wts": wts,
                        "xfix": xfixv.astype(ml_dtypes.bfloat16)})
    return in_maps


def _unshard(results):
    outs = []
    for i in range(NCORES):
        y = np.asarray(results[i]["yout"], dtype=np.float32)  # [128,2,16,256]
        img = y.transpose(2, 1, 0, 3).reshape(BPC, 1, P, P)
        ye = np.asarray(results[i]["yedge"], dtype=np.float32)
        ye = ye.reshape(128, 2, BPC, 2).transpose(2, 1, 0, 3)  # [b,m,p,e]
        img[:, 0, :, 0] = ye[:, :, :, 0].reshape(BPC, P)
        img[:, 0, :, P - 1] = ye[:, :, :, 1].reshape(BPC, P)
        yf = np.asarray(results[i]["yfix"], dtype=np.float32)  # [32, 256]
        yf = yf.reshape(2, BPC, P)
        img[:, 0, 127, :] = yf[0]
        img[:, 0, 128, :] = yf[1]
        outs.append(img)
    out = np.concatenate(outs, axis=0)
    out -= 1.0                    # source term f_s == 1, folded here
    return np.ascontiguousarray(out)


def _run(x0_pred, trace=False, tmpdir=None):
    import time
    from concourse.bass_utils import run_bass_kernel_spmd
    nc = _get_program()[0]
    in_maps = _shard_inputs(x0_pred)
    try:
        res = run_bass_kernel_spmd(nc, in_maps, list(range(NCORES)),
                                   trace=trace, tmpdir=tmpdir)
    except Exception:
        # transient NRT execution failures have been observed; one retry
        time.sleep(2.0)
        res = run_bass_kernel_spmd(nc, in_maps, list(range(NCORES)),
                                   trace=trace, tmpdir=tmpdir)
    return _unshard(res.results), res


def kernel(x0_pred):
    out, _ = _run(x0_pred, trace=False)
    return out
